# revision 6
# baseline (speedup 1.0000x reference)
"""Trainium2 SPMD kernel for nn_Attentionlayer_9208409883387.

Mathematical simplification: the reference computes
    h   = x @ W
    att = softmax(mask(leaky_relu(s1+s2), adj), axis=3)
    res = leaky_relu(h * sum_j att[..., j])
The row-sum of a softmax along its normalization axis is identically 1
(every row has >=1 unmasked entry: P[all-zero adj row] ~ 2^-1024), so
    res = leaky_relu(x @ W)
exactly, up to fp32 rounding of the softmax row-sum.

Strategy: data-parallel over the 48*1024 = 49152 rows, 6144 rows/core.
Each core's shard is laid out host-side with f_in on partitions
(rows[0:3072].T on partitions 0:64, rows[3072:6144].T on 64:128) so the
PE can consume it directly as the moving operand.  W is replicated as a
block-diagonal W (+) W [128,128] stationary operand.

I/O runs in bfloat16 both directions (halves the HBM traffic vs fp32;
measured end-to-end rel-l2 ~3e-3, well under the 2e-2 gate).  Each
input transfer is its own DRAM tensor so the HBM read is fully
sequential (strided 2KB-row reads measured ~150-250GB/s vs ~370GB/s
sequential).  The body is a 4-chunk pipeline: in-DMA chunk -> matmul
(PSUM fp32) -> leaky_relu -> out-DMA, with ACT handling chunks 0,1,3
and the DVE chunk 2 (max(x, 0.01x) via mul+max), and dummy matmuls
warming the PE's p-state ramp (0.65->1.2->2.4GHz) during the first
input-DMA wait.
"""

import numpy as np

B, T, N, F = 4, 12, 1024, 64
N_CORES = 8
ROWS = B * T * N              # 49152
RPC = ROWS // N_CORES         # 6144 rows per core
HALF = RPC // 2               # 3072 packed columns per core

_PROGRAM = None

# compute chunks in packed-column space: sizes 1024,1024,512,512
CHUNKS = [(0, 1024), (1024, 2048), (2048, 2560), (2560, 3072)]
N_WARM_MM = 3                 # dummy matmuls to ramp the PE p-state


def _build_program_raw():
    import concourse.bass as bass
    import concourse.mybir as mybir
    from contextlib import ExitStack

    f32 = mybir.dt.float32
    bf16 = mybir.dt.bfloat16
    nc = bass.Bass("TRN2")
    # One DRAM tensor per input transfer -> sequential HBM reads.
    # xp0 carries the 128-col block-diag W ahead of chunk 0.
    xps = [
        nc.declare_dram_parameter(
            f"xp{i}", [128, (hi - lo) + (128 if i == 0 else 0)], bf16, isOutput=False
        )
        for i, (lo, hi) in enumerate(CHUNKS)
    ]
    yp = nc.declare_dram_parameter("ypack", [128, HALF], bf16, isOutput=True)

    with ExitStack() as ctx:
        x_sb = ctx.enter_context(nc.sbuf_tensor("x_sb", [128, 128 + HALF], bf16))
        y_sb = ctx.enter_context(nc.sbuf_tensor("y_sb", [128, HALF], bf16))
        tmp = ctx.enter_context(nc.sbuf_tensor("tmp", [128, 512], f32))
        warm = ctx.enter_context(nc.sbuf_tensor("warm", [1, 4], f32))
        # banks 0-5 for real chunks, bank 6 as warm-up scratch
        ps = ctx.enter_context(nc.psum_tensor("ps", [128, HALF + 512], f32))
        din = [ctx.enter_context(nc.semaphore(f"din{i}")) for i in range(4)]
        pe_sem = ctx.enter_context(nc.semaphore("pe_sem"))
        act_sem = ctx.enter_context(nc.semaphore("act_sem"))
        dve_sem = ctx.enter_context(nc.semaphore("dve_sem"))
        dma_out = ctx.enter_context(nc.semaphore("dma_out"))
        block = ctx.enter_context(nc.Block())

        @block.sync
        def _(sync):
            # inputs (T0 = W + chunk 0)
            sync.dma_start(out=x_sb[:, 0:1152], in_=xps[0][:, :]).then_inc(din[0], 16)
            sync.dma_start(out=x_sb[:, 1152:2176], in_=xps[1][:, :]).then_inc(
                din[1], 16
            )
            sync.dma_start(out=x_sb[:, 2176:2688], in_=xps[2][:, :]).then_inc(
                din[2], 16
            )
            sync.dma_start(out=x_sb[:, 2688:3200], in_=xps[3][:, :]).then_inc(
                din[3], 16
            )
            # outputs in expected-completion order: c0+c1 (ACT), c2 (DVE), c3 (ACT)
            sync.wait_ge(act_sem, 2)
            sync.dma_start(out=yp[:, 0:2048], in_=y_sb[:, 0:2048]).then_inc(
                dma_out, 16
            )
            sync.wait_ge(dve_sem, 1)
            sync.dma_start(out=yp[:, 2048:2560], in_=y_sb[:, 2048:2560]).then_inc(
                dma_out, 16
            )
            sync.wait_ge(act_sem, 3)
            sync.dma_start(out=yp[:, 2560:3072], in_=y_sb[:, 2560:3072]).then_inc(
                dma_out, 16
            )
            sync.wait_ge(dma_out, 48)

        @block.tensor
        def _(tensor):
            w_ap = x_sb[:, 0:128]
            # p-state warm-up: garbage matmuls into scratch bank 6 while
            # the first input transfer is in flight.
            for _ in range(N_WARM_MM):
                nc.tensor.matmul(
                    ps[:, HALF : HALF + 512],
                    w_ap,
                    x_sb[:, 128:640],
                    start=True,
                    stop=True,
                )
            for i, (lo, hi) in enumerate(CHUNKS):
                tensor.wait_ge(din[i], 16)
                for mlo in range(lo, hi, 512):
                    nc.tensor.matmul(
                        ps[:, mlo : mlo + 512],
                        w_ap,
                        x_sb[:, 128 + mlo : 128 + mlo + 512],
                        start=True,
                        stop=True,
                    ).then_inc(pe_sem, 1)

        @block.scalar
        def _(scalar):
            # Touch the Lrelu table so walrus's lazy ACT_TABLE_LOAD (~1.3us)
            # runs during the DMA preamble, not before the first real ACT.
            nc.scalar.activation(
                warm[:, :], warm[:, :],
                mybir.ActivationFunctionType.Lrelu, alpha=0.01,
            )
            # ACT handles chunks 0, 1, 3; mm counts per chunk [2,2,1,1]
            for ci, need in ((0, 2), (1, 4), (3, 6)):
                lo, hi = CHUNKS[ci]
                scalar.wait_ge(pe_sem, need)
                nc.scalar.activation(
                    y_sb[:, lo:hi],
                    ps[:, lo:hi],
                    mybir.ActivationFunctionType.Lrelu,
                    alpha=0.01,
                ).then_inc(act_sem, 1)

        @block.vector
        def _(vector):
            # leaky_relu = max(x, 0.01x); DVE can read only one PSUM
            # operand per instruction, so stage 0.01x through SBUF.
            lo, hi = CHUNKS[2]
            vector.wait_ge(pe_sem, 5)
            nc.vector.tensor_scalar_mul(tmp[:, :], ps[:, lo:hi], 0.01)
            nc.vector.tensor_max(
                y_sb[:, lo:hi], ps[:, lo:hi], tmp[:, :]
            ).then_inc(dve_sem, 1)

    nc.finalize()
    return nc


_build_program = _build_program_raw


def _get_program():
    global _PROGRAM
    if _PROGRAM is None:
        _PROGRAM = _build_program()
    return _PROGRAM


def _make_in_maps(x, W):
    import ml_dtypes

    bf16 = ml_dtypes.bfloat16
    xr = np.ascontiguousarray(x, dtype=np.float32).reshape(N_CORES, RPC, F)
    wpack = np.zeros((128, 128), np.float32)
    wpack[0:64, 0:64] = W
    wpack[64:128, 64:128] = W
    wpack16 = wpack.astype(bf16)
    in_maps = []
    for c in range(N_CORES):
        xpack = np.empty((128, HALF), bf16)
        xpack[0:64] = xr[c, 0:HALF].T.astype(bf16)
        xpack[64:128] = xr[c, HALF:].T.astype(bf16)
        m = {}
        for i, (lo, hi) in enumerate(CHUNKS):
            if i == 0:
                m["xp0"] = np.ascontiguousarray(
                    np.concatenate([wpack16, xpack[:, lo:hi]], axis=1)
                )
            else:
                m[f"xp{i}"] = np.ascontiguousarray(xpack[:, lo:hi])
        in_maps.append(m)
    return in_maps


def run_spmd(x, W, **spmd_kwargs):
    """Run the Bass program on 8 cores; returns (y_full, BassKernelResults)."""
    from concourse.bass_utils import run_bass_kernel_spmd

    in_maps = _make_in_maps(x, W)
    res = run_bass_kernel_spmd(
        _get_program(), in_maps, list(range(N_CORES)), **spmd_kwargs
    )
    y = np.empty((N_CORES, RPC, F), np.float32)
    for c in range(N_CORES):
        ypack = np.asarray(res.results[c]["ypack"]).astype(np.float32)
        y[c, 0:HALF] = ypack[0:64].T
        y[c, HALF:] = ypack[64:128].T
    return y.reshape(B, T, N, F), res


def kernel(x, adj, W, a):
    # adj and a are mathematically dead (softmax row-sum == 1); see module doc.
    y, _ = run_spmd(np.asarray(x), np.asarray(W, dtype=np.float32))
    return y


# revision 7
# speedup vs baseline: 1.1067x; 1.1067x over previous
"""Trainium2 SPMD kernel for nn_Attentionlayer_9208409883387.

Mathematical simplification: the reference computes
    h   = x @ W
    att = softmax(mask(leaky_relu(s1+s2), adj), axis=3)
    res = leaky_relu(h * sum_j att[..., j])
The row-sum of a softmax along its normalization axis is identically 1
(every row has >=1 unmasked entry: P[all-zero adj row] ~ 2^-1024), so
    res = leaky_relu(x @ W)
exactly, up to fp32 rounding of the softmax row-sum.

Strategy: data-parallel over the 48*1024 = 49152 rows, 6144 rows/core.
Each core's shard is laid out host-side with f_in on partitions
(rows[0:3072].T on partitions 0:64, rows[3072:6144].T on 64:128) so the
PE can consume it directly as the moving operand.  W is replicated as a
block-diagonal W (+) W [128,128] stationary operand.

I/O runs in bfloat16 both directions (halves the HBM traffic vs fp32;
measured rel-l2 ~3e-3, well under the 2e-2 gate).  Each input transfer
is its own DRAM tensor so the HBM read is fully sequential (strided
2KB-row reads measured ~150-250GB/s vs ~370-400GB/s sequential).

Pipeline: 4 chunks (512/1024/512/1024 cols).  Input triggers split
across both HWDGE rings (SP issues T0,T2; the ACT engine issues T1,T3
before its activation work) so triggers and receipts overlap.  Matmuls
(PSUM fp32) chain on arrival; ACT applies Lrelu to chunks 0,1,3; the
DVE computes chunk 2 as max(x, 0.01x) (mul+max pair) in parallel with
ACT.  Four ready-gated output transfers stream per chunk from the SP
ring.  Dummy matmuls keep the PE busy during the first input wait
(p-state ramp 0.65->1.2->2.4GHz needs continuous busy time).
"""

import numpy as np

B, T, N, F = 4, 12, 1024, 64
N_CORES = 8
ROWS = B * T * N              # 49152
RPC = ROWS // N_CORES         # 6144 rows per core
HALF = RPC // 2               # 3072 packed columns per core

_PROGRAM = None

# compute chunks in packed-column space
CHUNKS = [(0, 512), (512, 1536), (1536, 2048), (2048, 3072)]
N_WARM_MM = 5                 # dummy matmuls to ramp the PE p-state


def _build_program_raw():
    import concourse.bass as bass
    import concourse.mybir as mybir
    from contextlib import ExitStack

    f32 = mybir.dt.float32
    bf16 = mybir.dt.bfloat16
    nc = bass.Bass("TRN2")
    # One DRAM tensor per input transfer -> sequential HBM reads.
    # xp0 carries the 128-col block-diag W ahead of chunk 0.
    xps = [
        nc.declare_dram_parameter(
            f"xp{i}", [128, (hi - lo) + (128 if i == 0 else 0)], bf16, isOutput=False
        )
        for i, (lo, hi) in enumerate(CHUNKS)
    ]
    yp = nc.declare_dram_parameter("ypack", [128, HALF], bf16, isOutput=True)

    with ExitStack() as ctx:
        x_sb = ctx.enter_context(nc.sbuf_tensor("x_sb", [128, 128 + HALF], bf16))
        y_sb = ctx.enter_context(nc.sbuf_tensor("y_sb", [128, HALF], bf16))
        tmp = ctx.enter_context(nc.sbuf_tensor("tmp", [128, 512], f32))
        warm = ctx.enter_context(nc.sbuf_tensor("warm", [1, 4], f32))
        # banks 0-5 for real chunks, bank 6 as warm-up scratch
        ps = ctx.enter_context(nc.psum_tensor("ps", [128, HALF + 512], f32))
        din = [ctx.enter_context(nc.semaphore(f"din{i}")) for i in range(4)]
        pe_sem = ctx.enter_context(nc.semaphore("pe_sem"))
        act_sem = ctx.enter_context(nc.semaphore("act_sem"))
        dve_sem = ctx.enter_context(nc.semaphore("dve_sem"))
        dma_out = ctx.enter_context(nc.semaphore("dma_out"))
        block = ctx.enter_context(nc.Block())

        # sbuf column ranges per transfer (W occupies 0:128)
        xsb_off = [(0, 640), (640, 1664), (1664, 2176), (2176, 3200)]

        @block.sync
        def _(sync):
            # inputs T0, T2 on the SP ring
            sync.dma_start(out=x_sb[:, 0:640], in_=xps[0][:, :]).then_inc(din[0], 16)
            sync.dma_start(out=x_sb[:, 1664:2176], in_=xps[2][:, :]).then_inc(
                din[2], 16
            )
            # outputs, gated per chunk in expected completion order
            sync.wait_ge(act_sem, 1)
            sync.dma_start(out=yp[:, 0:512], in_=y_sb[:, 0:512]).then_inc(dma_out, 16)
            sync.wait_ge(act_sem, 2)
            sync.dma_start(out=yp[:, 512:1536], in_=y_sb[:, 512:1536]).then_inc(
                dma_out, 16
            )
            sync.wait_ge(dve_sem, 1)
            sync.dma_start(out=yp[:, 1536:2048], in_=y_sb[:, 1536:2048]).then_inc(
                dma_out, 16
            )
            sync.wait_ge(act_sem, 3)
            sync.dma_start(out=yp[:, 2048:3072], in_=y_sb[:, 2048:3072]).then_inc(
                dma_out, 16
            )
            sync.wait_ge(dma_out, 64)

        @block.tensor
        def _(tensor):
            w_ap = x_sb[:, 0:128]
            # p-state warm-up: garbage matmuls into scratch bank 6 while
            # the first input transfer is in flight.
            for _ in range(N_WARM_MM):
                nc.tensor.matmul(
                    ps[:, HALF : HALF + 512],
                    w_ap,
                    x_sb[:, 128:640],
                    start=True,
                    stop=True,
                )
            for i, (lo, hi) in enumerate(CHUNKS):
                tensor.wait_ge(din[i], 16)
                for mlo in range(lo, hi, 512):
                    nc.tensor.matmul(
                        ps[:, mlo : mlo + 512],
                        w_ap,
                        x_sb[:, 128 + mlo : 128 + mlo + 512],
                        start=True,
                        stop=True,
                    ).then_inc(pe_sem, 1)

        @block.scalar
        def _(scalar):
            # inputs T1, T3 on the ACT ring, issued before compute work
            scalar.dma_start(out=x_sb[:, 640:1664], in_=xps[1][:, :]).then_inc(
                din[1], 16
            )
            scalar.dma_start(out=x_sb[:, 2176:3200], in_=xps[3][:, :]).then_inc(
                din[3], 16
            )
            # Touch the Lrelu table so walrus's lazy ACT_TABLE_LOAD (~1.3us)
            # runs during the DMA preamble, not before the first real ACT.
            nc.scalar.activation(
                warm[:, :], warm[:, :],
                mybir.ActivationFunctionType.Lrelu, alpha=0.01,
            )
            # ACT handles chunks 0, 1, 3; mm counts per chunk [1,2,1,2]
            for ci, need in ((0, 1), (1, 3), (3, 6)):
                lo, hi = CHUNKS[ci]
                scalar.wait_ge(pe_sem, need)
                nc.scalar.activation(
                    y_sb[:, lo:hi],
                    ps[:, lo:hi],
                    mybir.ActivationFunctionType.Lrelu,
                    alpha=0.01,
                ).then_inc(act_sem, 1)

        @block.vector
        def _(vector):
            # leaky_relu = max(x, 0.01x); DVE can read only one PSUM
            # operand per instruction, so stage 0.01x through SBUF.
            lo, hi = CHUNKS[2]
            vector.wait_ge(pe_sem, 4)
            nc.vector.tensor_scalar_mul(tmp[:, :], ps[:, lo:hi], 0.01)
            nc.vector.tensor_max(
                y_sb[:, lo:hi], ps[:, lo:hi], tmp[:, :]
            ).then_inc(dve_sem, 1)

    nc.finalize()
    return nc


_build_program = _build_program_raw


def _get_program():
    global _PROGRAM
    if _PROGRAM is None:
        _PROGRAM = _build_program()
    return _PROGRAM


def _make_in_maps(x, W):
    import ml_dtypes

    bf16 = ml_dtypes.bfloat16
    xr = np.ascontiguousarray(x, dtype=np.float32).reshape(N_CORES, RPC, F)
    wpack = np.zeros((128, 128), np.float32)
    wpack[0:64, 0:64] = W
    wpack[64:128, 64:128] = W
    wpack16 = wpack.astype(bf16)
    in_maps = []
    for c in range(N_CORES):
        xpack = np.empty((128, HALF), bf16)
        xpack[0:64] = xr[c, 0:HALF].T.astype(bf16)
        xpack[64:128] = xr[c, HALF:].T.astype(bf16)
        m = {}
        for i, (lo, hi) in enumerate(CHUNKS):
            if i == 0:
                m["xp0"] = np.ascontiguousarray(
                    np.concatenate([wpack16, xpack[:, lo:hi]], axis=1)
                )
            else:
                m[f"xp{i}"] = np.ascontiguousarray(xpack[:, lo:hi])
        in_maps.append(m)
    return in_maps


def run_spmd(x, W, **spmd_kwargs):
    """Run the Bass program on 8 cores; returns (y_full, BassKernelResults)."""
    from concourse.bass_utils import run_bass_kernel_spmd

    in_maps = _make_in_maps(x, W)
    res = run_bass_kernel_spmd(
        _get_program(), in_maps, list(range(N_CORES)), **spmd_kwargs
    )
    y = np.empty((N_CORES, RPC, F), np.float32)
    for c in range(N_CORES):
        ypack = np.asarray(res.results[c]["ypack"]).astype(np.float32)
        y[c, 0:HALF] = ypack[0:64].T
        y[c, HALF:] = ypack[64:128].T
    return y.reshape(B, T, N, F), res


def kernel(x, adj, W, a):
    # adj and a are mathematically dead (softmax row-sum == 1); see module doc.
    y, _ = run_spmd(np.asarray(x), np.asarray(W, dtype=np.float32))
    return y
